# revision 1
# baseline (speedup 1.0000x reference)
"""Trainium2 Bass kernel for the LIF/hh neuron module.

Math (from the reference):
  fc = x @ W_fc.T + b_fc                    [B, T, C]
  per step t (state mem[B,C,4], spike[B,C]):
    x4   = mem[...,:3] @ w + b              (old mem)
    keep = DECAY * (1 - spike)
    mem03' = mem[...,:3]*keep + fc_t        (channels 0..2 identical updates!)
    mem3'  = mem[...,3]*keep + x4
    mem1 = mem03' @ w + b + mem3'
    spike' = mem1 > THRESH

Key identity: channels 0..2 of mem start at 0 and receive identical updates,
so m0==m1==m2 =: m for all t.  Let W = w0+w1+w2, u := W*m + b.  Then with
v_t := W * fc_t (folded into the GEMM weights on host), and b==0:
    u'    = DECAY*(1-s)*u + v_t
    m3'   = DECAY*(1-s)*m3 + u
    mem1  = u' + m3'
    s'    = mem1 > THRESH
Further, mem1 = u + m3 after update, so with n := 1-s (inverted spike):
    w_t   = u + v_t                       (off critical chain)
    mem1' = DECAY*(mem1*n) + w_t          (3-op critical chain with n' below)
    u'    = DECAY*(u*n) + v_t             (off chain)
    n'    = (mem1' <= THRESH)
State: (u, mem1, n).  Verified bit-identical to the reference recurrence.

GEMM: single fp32r (FP22) pass.  The tensor engine runs fp32r at 1 cycle/row
for moving dim >= 256 (vs 3 bf16 hi/lo passes for the same accuracy class),
and fp22's 13-bit mantissa gives ~2^-13 relative error -> ~200 spike flips
(rel ~6e-3, gate 2e-2).

Sharding: data-parallel over batch, B=256 -> 32 per core on 8 cores.
Per-core GEMM: [C=4096, K=4096] x [K, N=480] with N = t*32+b.
Recurrence layout: partition p = c%128, free = j*32 + b (j = c//128), G=4
column groups; group g's recurrence overlaps the GEMM of later groups'
M-tiles, with chain ops on Vector and off-chain ops on GpSimd.
"""
import sys
import os

sys.path.insert(0, "/opt/trn_rl_repo")

import numpy as np
import ml_dtypes

THRESH = 0.8
DECAY = 0.2

B, T, IN, C = 256, 15, 4096, 4096
NCORES = 8
BL = B // NCORES          # 32 batch per core
N = BL * T                # 480 moving columns per core
KS = IN // 128            # 32 K-subtiles
JC = C // 128             # 32 c-chunks (M-tiles)
FREE = JC * BL            # 1024 state free dim
GROUPS = (8, 8, 8, 8)             # recurrence group widths in j-chunks
XCH = 8                   # x load chunks (startup pipelining)
KCH = KS // XCH           # K-subtiles per x chunk

LAST_EXEC_S = None
LAST_NC = None            # stashed Bass module for test harness profiling


def _numpy_fallback(x, W_fc, b_fc, W_lif, b_lif):
    fc = np.einsum("bti,ci->btc", x.astype(np.float64), W_fc.astype(np.float64))
    fc += b_fc.astype(np.float64)
    w = W_lif[0].astype(np.float64)
    b = float(b_lif[0])
    Bs, Ts, Cs = fc.shape
    mem = np.zeros((Bs, Cs, 4))
    spike = np.zeros((Bs, Cs))
    outs = []
    for t in range(Ts):
        x4 = mem[..., :3] @ w + b
        keep = DECAY * (1.0 - spike)
        mem03 = mem[..., :3] * keep[..., None] + fc[:, t][..., None]
        mem3 = mem[..., 3] * keep + x4
        mem = np.concatenate([mem03, mem3[..., None]], axis=-1)
        mem1 = mem03 @ w + b + mem3
        spike = (mem1 > THRESH).astype(np.float64)
        outs.append(spike)
    return np.stack(outs, axis=1).astype(x.dtype)


def _legalize_waits(nc, mybir):
    """Walrus codegen caps embedded sync-waits per instruction (Matmult: 1,
    DMACopy: 2, ...).  Tile's sem assignment can exceed that.  Engines and
    DMA sequencers execute their queues in order, so moving excess waits onto
    freshly inserted same-engine NoOps directly before the instruction is
    semantically identical.  One wait per NoOp (NoOp capacity unknown)."""
    limits = {}
    counter = [0]
    for fn in nc.m.functions:
        for blk in fn.blocks:
            insts = blk.instructions
            out = []
            changed = False
            for inst in insts:
                tname = type(inst).__name__
                lim = limits.get(tname, 1)
                si = inst.sync_info
                waits = list(si.on_wait) if si is not None else []
                if len(waits) > lim:
                    excess, kept = waits[:-lim], waits[-lim:]
                    for w in excess:
                        counter[0] += 1
                        out.append(mybir.InstNoOp(
                            name=f"WSPLIT-{counter[0]}",
                            engine=inst.engine,
                            ins=[], outs=[],
                            sync_info=mybir.SyncInfo(on_wait=[w], on_update=[]),
                        ))
                    inst.sync_info = mybir.SyncInfo(
                        on_wait=kept, on_update=list(si.on_update))
                    changed = True
                out.append(inst)
            if changed:
                blk.instructions = out
    return counter[0]


def _build_bass():
    import concourse.bass as bass
    import concourse.mybir as mybir
    import concourse.tile as tile
    from contextlib import ExitStack

    f32 = mybir.dt.float32
    f32r = mybir.dt.float32r
    Alu = mybir.AluOpType

    nc = bass.Bass()
    wt_d = nc.dram_tensor("wt", [JC, 128, KS, 128], f32r, kind="ExternalInput")
    xt_d = nc.dram_tensor("xt", [128, KS, N], f32r, kind="ExternalInput")
    sp_d = nc.dram_tensor("sp", [T, 128, FREE], f32, kind="ExternalOutput")

    with ExitStack() as ctx:
        tc = ctx.enter_context(tile.TileContext(nc))
        wpool = ctx.enter_context(tc.tile_pool(name="wpool", bufs=3))
        xpool = ctx.enter_context(tc.tile_pool(name="xpool", bufs=1))
        fcpool = ctx.enter_context(tc.tile_pool(name="fcpool", bufs=1))
        spool = ctx.enter_context(tc.tile_pool(name="state", bufs=1))
        ppool = ctx.enter_context(tc.tile_pool(name="psum", bufs=4, space="PSUM"))

        # x resident in SBUF, loaded in XCH chunks so the first matmuls
        # only wait on chunk 0; weight tiles stream per-M-tile (bufs=3
        # self-throttles the prefetch depth)
        x_tiles = []
        for ci in range(XCH):
            xtile = xpool.tile([128, KCH, N], f32r, tag=f"x{ci}", name=f"x{ci}")
            nc.sync.dma_start(xtile[:], xt_d[:, ci * KCH:(ci + 1) * KCH, :])
            x_tiles.append(xtile)

        fc_sbuf = fcpool.tile([128, T, FREE], f32)

        # per-group state tiles (distinct tags so group chains can overlap);
        # u and ns are double-buffered to avoid cross-engine WAR stalls
        st = []
        for g in range(len(GROUPS)):
            gf = GROUPS[g] * BL
            d = {nm: spool.tile([128, gf], f32, tag=f"{nm}{g}", name=f"{nm}{g}")
                 for nm in ("u", "uz", "w", "mem1", "z", "ns")}
            st.append(d)

        def emit_recurrence(g, j0):
            gf = GROUPS[g] * BL
            sl = slice(j0 * BL, j0 * BL + gf)
            d = st[g]
            u, uz, w = d["u"], d["uz"], d["w"]
            mem1, z, ns = d["mem1"], d["z"], d["ns"]
            nc.gpsimd.memset(u[:], 0.0)
            nc.vector.memset(mem1[:], 0.0)
            nc.vector.memset(ns[:], 1.0)
            for t in range(T):
                v_t = fc_sbuf[:, t, sl]
                # off-chain (GpSimd): w = u + v_t ; uz = u*ns
                nc.gpsimd.tensor_tensor(w[:], u[:], v_t, Alu.add)
                nc.gpsimd.tensor_tensor(uz[:], u[:], ns[:], Alu.mult)
                # chain (Vector): z = mem1*ns ; mem1' = D*z + w ; ns = mem1'<=th
                nc.vector.tensor_tensor(z[:], mem1[:], ns[:], Alu.mult)
                nc.vector.scalar_tensor_tensor(
                    u[:], uz[:], DECAY, v_t, Alu.mult, Alu.add)
                nc.vector.scalar_tensor_tensor(
                    mem1[:], z[:], DECAY, w[:], Alu.mult, Alu.add)
                nc.vector.tensor_scalar(
                    ns[:], mem1[:], THRESH, None, Alu.is_le)
                nc.sync.dma_start(sp_d[t, :, sl], ns[:])

        g, j0 = 0, 0
        for j in range(JC):
            wj = wpool.tile([128, KS, 128], f32r, tag="wj", name=f"w{j}")
            nc.sync.dma_start(wj[:], wt_d[j])
            ps = ppool.tile([128, N], f32)
            for s in range(KS):
                nc.tensor.matmul(
                    ps[:], wj[:, s, :], x_tiles[s // KCH][:, s % KCH, :],
                    start=(s == 0), stop=(s == KS - 1))
            nc.scalar.copy(
                fc_sbuf[:, :, j * BL:(j + 1) * BL],
                ps.rearrange("p (t b) -> p t b", b=BL))
            if j == j0 + GROUPS[g] - 1:
                emit_recurrence(g, j0)
                j0 += GROUPS[g]
                g += 1
    _legalize_waits(nc, mybir)
    return nc


_CACHE = {}


def _get_runner():
    """Compile once; return (fn, in_names, out_names, zero_outs, mesh)."""
    if "fn" in _CACHE:
        return _CACHE["fn"]
    global LAST_NC
    import jax
    import numpy as _np
    from jax.sharding import Mesh, PartitionSpec
    from jax.experimental.shard_map import shard_map
    import concourse.mybir as mybir
    from concourse import bass2jax

    bass2jax.install_neuronx_cc_hook()
    nc = _build_bass()
    LAST_NC = nc

    in_names, out_names, out_avals, zero_outs = [], [], [], []
    partition_name = nc.partition_id_tensor.name if nc.partition_id_tensor else None
    for alloc in nc.m.functions[0].allocations:
        if not isinstance(alloc, mybir.MemoryLocationSet):
            continue
        name = alloc.memorylocations[0].name
        if alloc.kind == "ExternalInput":
            if name != partition_name:
                in_names.append(name)
        elif alloc.kind == "ExternalOutput":
            shape = tuple(alloc.tensor_shape)
            dtype = mybir.dt.np(alloc.dtype)
            out_names.append(name)
            out_avals.append(jax.core.ShapedArray(shape, dtype))
            zero_outs.append(_np.zeros(shape, dtype))
    n_params = len(in_names)
    all_in_names = list(in_names) + list(out_names)
    if partition_name is not None:
        all_in_names.append(partition_name)
    donate = tuple(range(n_params, n_params + len(out_names)))

    def _body(*args):
        operands = list(args)
        if partition_name is not None:
            operands.append(bass2jax.partition_id_tensor())
        outs = bass2jax._bass_exec_p.bind(
            *operands,
            out_avals=tuple(out_avals),
            in_names=tuple(all_in_names),
            out_names=tuple(out_names),
            lowering_input_output_aliases=(),
            sim_require_finite=True,
            sim_require_nnan=True,
            nc=nc,
        )
        return tuple(outs)

    devices = jax.devices()[:NCORES]
    mesh = Mesh(_np.asarray(devices), ("core",))
    n_all = n_params + len(out_names)
    sharded = jax.jit(
        shard_map(_body, mesh=mesh,
                  in_specs=(PartitionSpec("core"),) * n_all,
                  out_specs=(PartitionSpec("core"),) * len(out_names),
                  check_rep=False),
        donate_argnums=donate, keep_unused=True,
    )
    _CACHE["fn"] = (sharded, in_names, out_names, zero_outs, mesh)
    return _CACHE["fn"]


def kernel(x, W_fc, b_fc, W_lif, b_lif):
    global LAST_EXEC_S
    if np.any(b_fc != 0) or np.any(b_lif != 0):
        return _numpy_fallback(x, W_fc, b_fc, W_lif, b_lif)
    import time
    import jax

    Ws = float(W_lif[0, 0]) + float(W_lif[0, 1]) + float(W_lif[0, 2])
    # lhsT layout: wt[j, p, s, m] = (Ws*W_fc).T[s*128+p, j*128+m]
    Wt = np.ascontiguousarray((W_fc.astype(np.float32) * np.float32(Ws)).T)
    wt = np.ascontiguousarray(
        Wt.reshape(KS, 128, JC, 128).transpose(2, 1, 0, 3))

    per_core = {"wt": [], "xt": []}
    for c in range(NCORES):
        xs = np.ascontiguousarray(
            x[c * BL:(c + 1) * BL].astype(np.float32).transpose(2, 1, 0)
        ).reshape(IN, N)  # [IN, t*BL+b]
        per_core["xt"].append(np.ascontiguousarray(
            xs.reshape(KS, 128, N).transpose(1, 0, 2)))
        per_core["wt"].append(wt)

    sharded, in_names, out_names, zero_outs, mesh = _get_runner()
    concat_in = [np.concatenate(per_core[n], axis=0) for n in in_names]
    concat_zero = [np.concatenate([z] * NCORES, axis=0) for z in zero_outs]

    from jax.sharding import NamedSharding, PartitionSpec
    shd = NamedSharding(mesh, PartitionSpec("core"))
    args = [jax.device_put(a, shd) for a in concat_in + concat_zero]
    for a in args:
        a.block_until_ready()
    t0 = time.time()
    out_arrs = sharded(*args)
    jax.block_until_ready(out_arrs)
    LAST_EXEC_S = time.time() - t0
    out_arrs = [np.asarray(o) for o in out_arrs]

    sp_all = out_arrs[out_names.index("sp")]            # [8*T, 128, FREE]
    out = np.empty((B, T, C), dtype=np.float32)
    for c in range(NCORES):
        sp = sp_all[c * T:(c + 1) * T]                  # [T, 128, FREE]
        arr = sp.reshape(T, 128, JC, BL)                # (t, p, j, b)
        spikes = 1.0 - np.transpose(arr, (3, 0, 2, 1))  # (b, t, j, p)
        out[c * BL:(c + 1) * BL] = spikes.reshape(BL, T, C)
    return out



# revision 2
# speedup vs baseline: 1.0942x; 1.0942x over previous
"""Trainium2 Bass kernel for the LIF/hh neuron module.

Math (from the reference):
  fc = x @ W_fc.T + b_fc                    [B, T, C]
  per step t (state mem[B,C,4], spike[B,C]):
    x4   = mem[...,:3] @ w + b              (old mem)
    keep = DECAY * (1 - spike)
    mem03' = mem[...,:3]*keep + fc_t        (channels 0..2 identical updates!)
    mem3'  = mem[...,3]*keep + x4
    mem1 = mem03' @ w + b + mem3'
    spike' = mem1 > THRESH

Key identity: channels 0..2 of mem start at 0 and receive identical updates,
so m0==m1==m2 =: m for all t.  Let W = w0+w1+w2, u := W*m + b.  Then with
v_t := W * fc_t (folded into the GEMM weights on host), and b==0:
    w_t   = u + v_t                       (off critical chain)
    mem1' = DECAY*(mem1*n) + w_t          (chain; n := 1-spike)
    u'    = DECAY*(u*n) + v_t             (off chain)
    n'    = (mem1' <= THRESH)
State: (u, mem1, n).  Verified bit-identical to the reference recurrence.

GEMM in fp32r (fp22): 1 cycle/row for moving dim >= 256; ~2^-13 relative
error -> rel ~1e-2 on spikes (gate 2e-2).

Sharding (v2): 2-way tensor-parallel over out channels x 4-way data-parallel
over batch.  Core c: channel half h=c//4 (2048 ch), batch quarter q=c%4
(64 samples).  Per-core HBM traffic: W 32MB + x 15MB + out 7.5MB = 55MB
(~180us) vs the 205us tensor floor -> tensor-bound (the v1 bs=8 layout read
75MB/core and was DMA-bound).

Per-core GEMM: M=2048 (16 M-tiles), K=4096 (32 subtiles), N=960 (col=t*64+b),
psum split 512+448 (t-aligned: 8t/7t).  x (15MB) resident, streamed in 16
K-pieces on the sync queue; W streamed as 32 half-tiles [128,16,128] (1MB)
through a 5-slot ring on the scalar queue.  Phase 1 (while x lands): matmuls
emitted piece-major across j0..j3 so the PE consumes x pieces as they
arrive (~51us of work vs ~55us x load).  Phase 2: j-major, weight ring
stays 2 tiles ahead.

Recurrence groups (4,4,4,2,1,1) M-tiles; each group's 15-step chain overlaps
the next group's GEMM.  Big groups: chain ops (z, mem1', ns) + u' on Vector,
off-chain (w, uz) on GpSimd.  The two 1-tile tail groups get their own
64-wide state tiles and run all ops on Vector (no cross-engine sems) to
minimize the end-of-kernel tail.
"""
import sys
import os

sys.path.insert(0, "/opt/trn_rl_repo")

import numpy as np
import ml_dtypes

THRESH = 0.8
DECAY = 0.2

B, T, IN, C = 256, 15, 4096, 4096
NCORES = 8
CS = 2                    # channel shards
BS = 4                    # batch shards
MC = C // CS              # 2048 channels per core
BL = B // BS              # 64 samples per core
N = BL * T                # 960 moving columns (col = t*64 + b)
KS = IN // 128            # 32 K-subtiles
JC = MC // 128            # 16 M-tiles per core
NPIECE = 16               # x K-pieces of 2 subtiles each
NA, NB = 512, 448         # psum split: t 0..7 | t 8..14
GROUPS = (4, 4, 4, 2, 1, 1)   # recurrence group sizes in M-tiles

LAST_EXEC_S = None
LAST_NC = None            # stashed Bass module for test harness profiling


def _numpy_fallback(x, W_fc, b_fc, W_lif, b_lif):
    fc = np.einsum("bti,ci->btc", x.astype(np.float64), W_fc.astype(np.float64))
    fc += b_fc.astype(np.float64)
    w = W_lif[0].astype(np.float64)
    b = float(b_lif[0])
    Bs, Ts, Cs = fc.shape
    mem = np.zeros((Bs, Cs, 4))
    spike = np.zeros((Bs, Cs))
    outs = []
    for t in range(Ts):
        x4 = mem[..., :3] @ w + b
        keep = DECAY * (1.0 - spike)
        mem03 = mem[..., :3] * keep[..., None] + fc[:, t][..., None]
        mem3 = mem[..., 3] * keep + x4
        mem = np.concatenate([mem03, mem3[..., None]], axis=-1)
        mem1 = mem03 @ w + b + mem3
        spike = (mem1 > THRESH).astype(np.float64)
        outs.append(spike)
    return np.stack(outs, axis=1).astype(x.dtype)


def _legalize_waits(nc, mybir):
    """Walrus codegen caps embedded sync-waits per instruction (Matmult: 1,
    DMACopy: 2, ...).  Tile's sem assignment can exceed that.  Engines and
    DMA sequencers execute their queues in order, so moving excess waits onto
    freshly inserted same-engine NoOps directly before the instruction is
    semantically identical.  One wait per NoOp (NoOp capacity unknown)."""
    limits = {}
    counter = [0]
    for fn in nc.m.functions:
        for blk in fn.blocks:
            insts = blk.instructions
            out = []
            changed = False
            for inst in insts:
                tname = type(inst).__name__
                lim = limits.get(tname, 1)
                si = inst.sync_info
                waits = list(si.on_wait) if si is not None else []
                if len(waits) > lim:
                    excess, kept = waits[:-lim], waits[-lim:]
                    for w in excess:
                        counter[0] += 1
                        out.append(mybir.InstNoOp(
                            name=f"WSPLIT-{counter[0]}",
                            engine=inst.engine,
                            ins=[], outs=[],
                            sync_info=mybir.SyncInfo(on_wait=[w], on_update=[]),
                        ))
                    inst.sync_info = mybir.SyncInfo(
                        on_wait=kept, on_update=list(si.on_update))
                    changed = True
                out.append(inst)
            if changed:
                blk.instructions = out
    return counter[0]


def _build_bass():
    import concourse.bass as bass
    import concourse.mybir as mybir
    import concourse.tile as tile
    from contextlib import ExitStack

    f32 = mybir.dt.float32
    f32r = mybir.dt.float32r
    Alu = mybir.AluOpType

    nc = bass.Bass()
    wt_d = nc.dram_tensor("wt", [JC, 2, 128, 16, 128], f32r, kind="ExternalInput")
    xt_d = nc.dram_tensor("xt", [128, KS, N], f32r, kind="ExternalInput")
    sp_d = nc.dram_tensor("sp", [128, T, JC * BL], f32, kind="ExternalOutput")

    # group bookkeeping: j -> group index at start/end
    gstart = {}
    gend = {}
    j0 = 0
    for g, sz in enumerate(GROUPS):
        gstart[j0] = g
        gend[j0 + sz - 1] = g
        j0 += sz
    goffs = np.cumsum([0] + list(GROUPS))[:-1] * BL

    with ExitStack() as ctx:
        tc = ctx.enter_context(tile.TileContext(nc))
        xpool = ctx.enter_context(tc.tile_pool(name="xpool", bufs=1))
        wpool = ctx.enter_context(tc.tile_pool(name="wpool", bufs=5))
        fcpool = ctx.enter_context(tc.tile_pool(name="fcpool", bufs=2))
        spool = ctx.enter_context(tc.tile_pool(name="state", bufs=1))
        ppool = ctx.enter_context(tc.tile_pool(name="psum", bufs=4, space="PSUM"))

        # x resident, streamed in 16 K-pieces on the sync queue
        xp = []
        for i in range(NPIECE):
            t_ = xpool.tile([128, 2, N], f32r, tag=f"x{i}", name=f"x{i}")
            nc.sync.dma_start(t_[:], xt_d[:, 2 * i:2 * i + 2, :])
            xp.append(t_)

        # weight half-tiles through a 5-slot ring on the scalar queue
        wh = {}

        def load_wh(j, hf):
            t_ = wpool.tile([128, 16, 128], f32r, tag="wh", name=f"w{j}h{hf}")
            nc.scalar.dma_start(t_[:], wt_d[j, hf])
            wh[(j, hf)] = t_

        for j in range(4):
            load_wh(j, 0)
        for j in range(4):
            load_wh(j, 1)
        load_wh(4, 0)
        load_wh(4, 1)

        # shared state tiles (big groups, <=256 wide) + per-tail-group tiles
        def state_set(pref, width):
            names = ("m1", "z", "uz", "w", "u0", "u1", "ns0", "ns1")
            return {nm: spool.tile([128, width], f32, tag=f"{pref}{nm}",
                                   name=f"{pref}{nm}") for nm in names}

        big_st = state_set("b_", 4 * BL)
        tail_st = {4: state_set("t4_", BL), 5: state_set("t5_", BL)}

        def emit_rec(g, fc_g, goff, gw, st, tail):
            m1 = st["m1"][:, :gw]
            z = st["z"][:, :gw]
            uz = st["uz"][:, :gw]
            w = st["w"][:, :gw]
            u_ = (st["u0"][:, :gw], st["u1"][:, :gw])
            ns_ = (st["ns0"][:, :gw], st["ns1"][:, :gw])
            v0 = fc_g[:, 0, :]
            nc.vector.tensor_scalar_add(m1, v0, 0.0)
            nc.vector.tensor_scalar_add(u_[0], v0, 0.0)
            nc.vector.tensor_scalar(ns_[0], m1, THRESH, None, Alu.is_le)
            nc.sync.dma_start(sp_d[:, 0, goff:goff + gw], ns_[0])
            for t in range(1, T):
                p, pp = t % 2, (t - 1) % 2
                vt = fc_g[:, t, :]
                if tail:
                    nc.vector.tensor_tensor(w, u_[pp], vt, Alu.add)
                    nc.vector.tensor_tensor(z, m1, ns_[pp], Alu.mult)
                    nc.vector.scalar_tensor_tensor(
                        m1, z, DECAY, w, Alu.mult, Alu.add)
                    nc.vector.tensor_scalar(ns_[p], m1, THRESH, None, Alu.is_le)
                    nc.vector.tensor_tensor(uz, u_[pp], ns_[pp], Alu.mult)
                    nc.vector.scalar_tensor_tensor(
                        u_[p], uz, DECAY, vt, Alu.mult, Alu.add)
                else:
                    nc.gpsimd.tensor_tensor(w, u_[pp], vt, Alu.add)
                    nc.gpsimd.tensor_tensor(uz, u_[pp], ns_[pp], Alu.mult)
                    nc.vector.tensor_tensor(z, m1, ns_[pp], Alu.mult)
                    nc.vector.scalar_tensor_tensor(
                        m1, z, DECAY, w, Alu.mult, Alu.add)
                    nc.vector.tensor_scalar(ns_[p], m1, THRESH, None, Alu.is_le)
                    nc.vector.scalar_tensor_tensor(
                        u_[p], uz, DECAY, vt, Alu.mult, Alu.add)
                nc.sync.dma_start(sp_d[:, t, goff:goff + gw], ns_[p])

        # phase-1 matmuls: piece-major over j0..j3 so the PE stream consumes
        # x pieces in arrival order (PE executes its queue in order)
        ps = {}
        for j in range(4):
            ps[j] = (ppool.tile([128, NA], f32, tag="psA", name=f"psA{j}"),
                     ppool.tile([128, NB], f32, tag="psB", name=f"psB{j}"))
        for i in range(NPIECE):
            hf = 0 if i < 8 else 1
            for j in range(4):
                for k in (0, 1):
                    s = 2 * i + k
                    lhsT = wh[(j, hf)][:, s - 16 * hf, :]
                    rhs = xp[i][:, k, :]
                    nc.tensor.matmul(ps[j][0], lhsT, rhs[:, 0:NA],
                                     start=(s == 0), stop=(s == KS - 1))
                    nc.tensor.matmul(ps[j][1], lhsT, rhs[:, NA:N],
                                     start=(s == 0), stop=(s == KS - 1))

        fc_cur = None
        cur_g = 0
        for j in range(JC):
            if j in gstart:
                cur_g = gstart[j]
                gw = GROUPS[cur_g] * BL
                fc_cur = fcpool.tile([128, T, gw], f32, tag="fc",
                                     name=f"fc{cur_g}")
            if j >= 4:
                psA = ppool.tile([128, NA], f32, tag="psA", name=f"psA{j}")
                psB = ppool.tile([128, NB], f32, tag="psB", name=f"psB{j}")
                ps[j] = (psA, psB)
                for s in range(KS):
                    hf = s // 16
                    lhsT = wh[(j, hf)][:, s % 16, :]
                    rhs = xp[s // 2][:, s % 2, :]
                    nc.tensor.matmul(psA, lhsT, rhs[:, 0:NA],
                                     start=(s == 0), stop=(s == KS - 1))
                    nc.tensor.matmul(psB, lhsT, rhs[:, NA:N],
                                     start=(s == 0), stop=(s == KS - 1))
            jj = j * BL - int(goffs[cur_g])
            psA, psB = ps[j]
            nc.scalar.copy(fc_cur[:, 0:8, jj:jj + BL],
                           psA.rearrange("p (t b) -> p t b", b=BL))
            nc.scalar.copy(fc_cur[:, 8:T, jj:jj + BL],
                           psB.rearrange("p (t b) -> p t b", b=BL))
            if j in gend:
                g = gend[j]
                emit_rec(g, fc_cur, int(goffs[g]), GROUPS[g] * BL,
                         tail_st.get(g, big_st), g >= 4)
            if 3 <= j <= 13:
                load_wh(j + 2, 0)
                load_wh(j + 2, 1)
    _legalize_waits(nc, mybir)
    return nc


_CACHE = {}


def _get_runner():
    """Compile once; return (fn, in_names, out_names, zero_outs, mesh)."""
    if "fn" in _CACHE:
        return _CACHE["fn"]
    global LAST_NC
    import jax
    import numpy as _np
    from jax.sharding import Mesh, PartitionSpec
    from jax.experimental.shard_map import shard_map
    import concourse.mybir as mybir
    from concourse import bass2jax

    bass2jax.install_neuronx_cc_hook()
    nc = _build_bass()
    LAST_NC = nc

    in_names, out_names, out_avals, zero_outs = [], [], [], []
    partition_name = nc.partition_id_tensor.name if nc.partition_id_tensor else None
    for alloc in nc.m.functions[0].allocations:
        if not isinstance(alloc, mybir.MemoryLocationSet):
            continue
        name = alloc.memorylocations[0].name
        if alloc.kind == "ExternalInput":
            if name != partition_name:
                in_names.append(name)
        elif alloc.kind == "ExternalOutput":
            shape = tuple(alloc.tensor_shape)
            dtype = mybir.dt.np(alloc.dtype)
            out_names.append(name)
            out_avals.append(jax.core.ShapedArray(shape, dtype))
            zero_outs.append(_np.zeros(shape, dtype))
    n_params = len(in_names)
    all_in_names = list(in_names) + list(out_names)
    if partition_name is not None:
        all_in_names.append(partition_name)
    donate = tuple(range(n_params, n_params + len(out_names)))

    def _body(*args):
        operands = list(args)
        if partition_name is not None:
            operands.append(bass2jax.partition_id_tensor())
        outs = bass2jax._bass_exec_p.bind(
            *operands,
            out_avals=tuple(out_avals),
            in_names=tuple(all_in_names),
            out_names=tuple(out_names),
            lowering_input_output_aliases=(),
            sim_require_finite=True,
            sim_require_nnan=True,
            nc=nc,
        )
        return tuple(outs)

    devices = jax.devices()[:NCORES]
    mesh = Mesh(_np.asarray(devices), ("core",))
    n_all = n_params + len(out_names)
    sharded = jax.jit(
        shard_map(_body, mesh=mesh,
                  in_specs=(PartitionSpec("core"),) * n_all,
                  out_specs=(PartitionSpec("core"),) * len(out_names),
                  check_rep=False),
        donate_argnums=donate, keep_unused=True,
    )
    _CACHE["fn"] = (sharded, in_names, out_names, zero_outs, mesh)
    return _CACHE["fn"]


def kernel(x, W_fc, b_fc, W_lif, b_lif):
    global LAST_EXEC_S
    if np.any(b_fc != 0) or np.any(b_lif != 0):
        return _numpy_fallback(x, W_fc, b_fc, W_lif, b_lif)
    import time
    import jax

    Ws = float(W_lif[0, 0]) + float(W_lif[0, 1]) + float(W_lif[0, 2])
    # lhsT layout: wt[j, hf, p, sh, m] = (Ws*W_fc).T[(hf*16+sh)*128+p,
    #                                               h*2048 + j*128 + m]
    Wt = np.ascontiguousarray((W_fc.astype(np.float32) * np.float32(Ws)).T)
    wt_h = []
    for h in range(CS):
        arr = Wt[:, h * MC:(h + 1) * MC].reshape(2, 16, 128, JC, 128)
        wt_h.append(np.ascontiguousarray(arr.transpose(3, 0, 2, 1, 4)))

    per_core = {"wt": [], "xt": []}
    for c in range(NCORES):
        h, q = c // BS, c % BS
        xs = np.ascontiguousarray(
            x[q * BL:(q + 1) * BL].astype(np.float32).transpose(2, 1, 0)
        ).reshape(KS, 128, N)  # [s, p, t*BL+b]
        per_core["xt"].append(np.ascontiguousarray(xs.transpose(1, 0, 2)))
        per_core["wt"].append(wt_h[h])

    sharded, in_names, out_names, zero_outs, mesh = _get_runner()
    concat_in = [np.concatenate(per_core[n], axis=0) for n in in_names]
    concat_zero = [np.concatenate([z] * NCORES, axis=0) for z in zero_outs]

    from jax.sharding import NamedSharding, PartitionSpec
    shd = NamedSharding(mesh, PartitionSpec("core"))
    args = [jax.device_put(a, shd) for a in concat_in + concat_zero]
    for a in args:
        a.block_until_ready()
    t0 = time.time()
    out_arrs = sharded(*args)
    jax.block_until_ready(out_arrs)
    LAST_EXEC_S = time.time() - t0
    out_arrs = [np.asarray(o) for o in out_arrs]

    sp_all = out_arrs[out_names.index("sp")]            # [8*128, T, JC*BL]
    out = np.empty((B, T, C), dtype=np.float32)
    for c in range(NCORES):
        h, q = c // BS, c % BS
        sp = sp_all[c * 128:(c + 1) * 128]              # [128, T, JC*BL]
        arr = sp.reshape(128, T, JC, BL)                # (p, t, j, b)
        spikes = 1.0 - np.transpose(arr, (3, 1, 2, 0))  # (b, t, j, p)
        out[q * BL:(q + 1) * BL, :, h * MC:(h + 1) * MC] = \
            spikes.reshape(BL, T, MC)
    return out
